# revision 1
# baseline (speedup 1.0000x reference)
"""Trainium2 Bass kernel for nn_Attention_30305289240928.

Single-layer causal attention with RMSNorm prologue:
    xn = x * rsqrt(mean(x^2) + eps)           (RMSNorm, no weight)
    qkv = xn @ wqkv.T  -> per-head q, k, v    (16 heads, head_dim 128)
    out = softmax(causal(q k^T / sqrt(128))) v, concat heads, @ wo.T

Sharding: head-parallel tensor parallel over 8 NeuronCores.
Core c owns heads 2c, 2c+1 (wqkv rows c*768:(c+1)*768) and the matching
wo input-columns c*256:(c+1)*256. Each core computes a full-shape partial
of the output projection (rank-256 contribution); the host sums the 8
partials (the TP all-reduce, done host-side at gather time).

Device-side design:
  - All matmuls in float32r (TF32-like, full PE rate at N>=256);
    measured end-to-end relative error ~3e-4.
  - The RMSNorm scale s[t] factors out of the projection: QKV is computed
    from RAW x, then s is folded into Q (free-dim broadcast multiply at
    PSUM eviction), into the exp() per-partition scale (s[kt]/sqrt(D)),
    and into V (per-partition multiply at eviction).
  - Scores are computed transposed, S.T[kt, qt], so the softmax-exp output
    feeds the PV matmul directly (kt on partitions) with no transposes.
    Causal masking = per-block N-sliced matmuls + one 128x128 triangular
    multiplicative mask on diagonal blocks; below-diagonal blocks are
    never computed.
  - sum-of-exp via ones-matmul accumulated in PSUM alongside PV;
    1/sumexp via single-pass Newton reciprocal on DVE.
  - DMA instruction count is managed against HWDGE descriptor-gen time
    (~0.6us/instruction): per-chunk DMAs only for the latency-critical
    tb=0 ramp, half-block batches for later xt loads, and grouped 2-row-
    block output writes. Output projection is interleaved one query-block
    behind attention so the softmax-normalize chain and the 16.8MB output
    DMA stay off the TensorE critical path.
"""

import numpy as np

import concourse.bacc as bacc
import concourse.mybir as mybir
import concourse.tile as tile
from concourse import bass_utils

# Problem shapes (hardcoded per contract)
S = 2048          # sequence length
H = 2048          # hidden
NH = 16           # heads
D = 128           # head dim
EPS = 1e-5
N_CORES = 8
HPC = NH // N_CORES        # heads per core = 2
FPC = 3 * D * HPC          # wqkv features per core = 768
CPC = D * HPC              # attn dims (wo input cols) per core = 256

TB = 256                   # token block width (phase 1)
NTB = S // TB              # 8
NM = TB // 128             # 128-wide sub-blocks per token block
NHO = H // 128             # 16 hidden 128-chunks
QB = 512                   # query block width (phase 2)
NQB = S // QB              # 4
NKB = S // 128             # 16 key 128-blocks
SQRT_D_INV = 1.0 / float(np.sqrt(D))

f32 = mybir.dt.float32
f32r = mybir.dt.float32r

_CACHED_NC = None


def _build():
    nc = bacc.Bacc("TRN2", target_bir_lowering=False, debug=False,
                   num_devices=N_CORES)
    xT_d = nc.dram_tensor("xT", [H, S], f32, kind="ExternalInput").ap()
    wT_d = nc.dram_tensor("wT", [H, FPC], f32, kind="ExternalInput").ap()
    woT_d = nc.dram_tensor("woT", [CPC, S], f32, kind="ExternalInput").ap()
    # cst = [ones(128,128) | zeros(128,128) | tri_upper(128,128) | eye(128,128)]
    cst_d = nc.dram_tensor("cst", [128, 512], f32, kind="ExternalInput").ap()
    outT_d = nc.dram_tensor("outT", [H, S], f32, kind="ExternalOutput").ap()

    with tile.TileContext(nc) as tc:
        with tc.tile_pool(name="const", bufs=1) as const_pool, \
             tc.tile_pool(name="qk", bufs=1) as qk_pool, \
             tc.tile_pool(name="vsb", bufs=1) as v_pool, \
             tc.tile_pool(name="attn", bufs=1) as attn_pool, \
             tc.tile_pool(name="svec", bufs=1) as s_pool:

            ones_r = const_pool.tile([128, 128], f32r, tag="ones")
            zt = const_pool.tile([128, 256], f32, tag="zt")   # [zeros | tri]
            tri = zt[:, 128:256]
            eye = const_pool.tile([128, 128], f32, tag="eye")
            eps_b = const_pool.tile([128, 1], f32, tag="eps")
            nc.gpsimd.memset(eps_b[:], EPS)

            # phase-1 outputs (live into phases 2/3)
            qkT = qk_pool.tile([128, 2 * HPC, S], f32r)   # [q0,k0,q1,k1] x S
            v_sb = v_pool.tile([128, NKB, CPC], f32r)     # V natural, t-chunked
            attnT = attn_pool.tile([128, HPC, S], f32r)   # O.T rows (this core)
            s_bc = s_pool.tile([128, NTB, TB], f32)       # s[t] bcast over parts
            sTd = s_pool.tile([128, NKB], f32)            # s[t]/sqrt(D), t on parts
            sT = s_pool.tile([128, NKB], f32)             # s[t] plain, t on parts

            # ---------------- Phase 1: RMSNorm stats + QKV projection ------
            with tc.tile_pool(name="wt", bufs=1) as wt_pool, \
                 tc.tile_pool(name="xt", bufs=2) as xt_pool, \
                 tc.tile_pool(name="sq", bufs=3) as sq_pool, \
                 tc.tile_pool(name="ph1", bufs=2) as ph1_pool, \
                 tc.tile_pool(name="ps_qk", bufs=4, space="PSUM") as psum_qk, \
                 tc.tile_pool(name="ps_v", bufs=2, space="PSUM") as psum_v, \
                 tc.tile_pool(name="ps_ssq", bufs=1, space="PSUM") as psum_ssq, \
                 tc.tile_pool(name="ps_t", bufs=1, space="PSUM") as psum_t:

                def load_xt(tb):
                    # two half-batched DMAs per token block: few HWDGE
                    # descriptor-gen slots, but the first half still lands
                    # early enough to start the ho-serial chains
                    chunks = []
                    for half in range(2):
                        t = xt_pool.tile([128, NHO // 2, TB], f32r,
                                         tag=f"xtb{half}")
                        nc.sync.dma_start(
                            t[:],
                            xT_d[half * 1024:(half + 1) * 1024,
                                 tb * TB:(tb + 1) * TB]
                            .rearrange("(ho p) t -> p ho t", p=128)
                            .bitcast(f32r))
                        chunks.extend(t[:, ho] for ho in range(NHO // 2))
                    return chunks

                # interleave xt(tb=0) and wt chunk loads so the first
                # K-matmul chain is DMA-paced with minimal lead time
                xt_cur = []
                wt = []
                for ho in range(NHO):
                    tx = wt_pool.tile([128, TB], f32r, tag=f"xt0_{ho}")
                    nc.sync.dma_start(
                        tx[:], xT_d[ho * 128:(ho + 1) * 128, 0:TB].bitcast(f32r))
                    xt_cur.append(tx)
                    tw = wt_pool.tile([128, FPC], f32r, tag=f"wt{ho}")
                    nc.sync.dma_start(
                        tw[:], wT_d[ho * 128:(ho + 1) * 128, :].bitcast(f32r))
                    wt.append(tw)
                    if ho == 1:
                        # only the ones tile is needed early (ssq matmuls)
                        nc.sync.dma_start(ones_r[:], cst_d[:, 0:128].bitcast(f32r))
                    if ho == NHO - 1:
                        # mask/identity consts are first used at the tb0
                        # transposes / phase 2 — keep them out of the ramp
                        nc.sync.dma_start(zt[:], cst_d[:, 128:384])
                        nc.sync.dma_start(eye[:], cst_d[:, 384:512])
                for tb in range(NTB):
                    xt = xt_cur
                    if tb + 1 < NTB:
                        xt_next = load_xt(tb + 1)

                    # squares first: ACT/DVE fill while PE runs K matmuls
                    sqs = []
                    for ho in range(NHO):
                        sq = sq_pool.tile([128, TB], f32r, tag=f"sq{ho % 4}")
                        if ho % 2 == 0:
                            nc.scalar.activation(
                                sq[:], xt[ho][:],
                                mybir.ActivationFunctionType.Square)
                        else:
                            nc.vector.tensor_tensor(
                                sq[:], xt[ho][:].bitcast(f32),
                                xt[ho][:].bitcast(f32), mybir.AluOpType.mult)
                        sqs.append(sq)

                    def qk_block(slot, fb):
                        # qkT slots: 0=q0 1=k0 2=q1 3=k1 ; feature layout per
                        # head: [q(128) k(128) v(128)] x 2 heads
                        ps = psum_qk.tile([128, TB], f32)
                        for ho in range(NHO):
                            nc.tensor.matmul(
                                ps[:], wt[ho][:, fb * 128:(fb + 1) * 128],
                                xt[ho][:], start=(ho == 0), stop=(ho == NHO - 1))
                        dst = qkT[:, slot, tb * TB:(tb + 1) * TB]
                        if slot in (0, 2):   # Q: scale by s[t] during eviction
                            nc.vector.tensor_tensor(dst, ps[:], s_bc[:, tb],
                                                    mybir.AluOpType.mult)
                        else:                # K: plain copy
                            nc.scalar.copy(dst, ps[:])

                    # K blocks (eviction independent of s)
                    qk_block(1, 1)
                    qk_block(3, 4)

                    # sum of squares over hidden (sq tiles all ready by now)
                    ps_ssq = psum_ssq.tile([128, TB], f32)
                    for ho in range(NHO):
                        nc.tensor.matmul(ps_ssq[:], ones_r[:], sqs[ho][:],
                                         start=(ho == 0), stop=(ho == NHO - 1))
                    # s = 1/sqrt(ssq/H + eps)
                    sqrt_t = ph1_pool.tile([128, TB], f32, tag="sqrt")
                    nc.scalar.activation(sqrt_t[:], ps_ssq[:],
                                         mybir.ActivationFunctionType.Sqrt,
                                         bias=eps_b[:], scale=1.0 / H)
                    nc.vector.reciprocal_approx_fast(s_bc[:, tb], sqrt_t[:])

                    # Q blocks (eviction waits on s_bc, ready by now)
                    qk_block(0, 0)
                    qk_block(2, 3)

                    # transpose s into partition-major sT/sTd columns (late:
                    # keeps the ssq->sqrt->recip latency off PE's back)
                    for m in range(NM):
                        pt = psum_t.tile([128, 128], f32)
                        nc.tensor.transpose(pt[:], s_bc[:, tb, m * 128:(m + 1) * 128],
                                            eye[:])
                        col = tb * NM + m
                        nc.scalar.mul(sTd[:, col:col + 1], pt[:, 0:1], SQRT_D_INV)
                        nc.scalar.copy(sT[:, col:col + 1], pt[:, 0:1])

                    # V blocks: out (t, dv) via lhsT = xT chunk, rhs = wv cols
                    for m in range(NM):
                        ps = psum_v.tile([128, CPC], f32)
                        for ho in range(NHO):
                            wv = wt[ho][:].rearrange(
                                "p (hd c f) -> p hd c f", hd=HPC, c=3)[:, :, 2, :]
                            nc.tensor.matmul(
                                ps[:], xt[ho][:, m * 128:(m + 1) * 128],
                                wv, start=(ho == 0), stop=(ho == NHO - 1))
                        chunk = tb * NM + m
                        nc.vector.tensor_scalar_mul(
                            v_sb[:, chunk], ps[:], sT[:, chunk:chunk + 1])

                    if tb + 1 < NTB:
                        xt_cur = xt_next

            # -------- Phase 2+3: attention (qb-outer) + output projection ---
            with tc.tile_pool(name="wo", bufs=1) as wo_pool, \
                 tc.tile_pool(name="exps", bufs=8) as exp_pool, \
                 tc.tile_pool(name="rse", bufs=2) as rse_pool, \
                 tc.tile_pool(name="ostage", bufs=8) as out_pool, \
                 tc.tile_pool(name="ps_s", bufs=3, space="PSUM") as psum_s, \
                 tc.tile_pool(name="ps_o", bufs=2, space="PSUM") as psum_o, \
                 tc.tile_pool(name="ps_se", bufs=1, space="PSUM") as psum_se, \
                 tc.tile_pool(name="ps_out", bufs=2, space="PSUM") as psum_out:
                # wo.T streams in while early attention runs (first use is
                # the qb=0 output-projection block, ~10us into phase 2)
                woT = wo_pool.tile([128, HPC, S], f32r)   # wo.T slice
                nc.sync.dma_start(
                    woT[:], woT_d.rearrange("(ch p) o -> p ch o", p=128)
                    .bitcast(f32r))
                def attn_head(qb, h):
                    kb_hi = (qb + 1) * (QB // 128) - 1
                    if True:
                        q_slot, k_slot = 2 * h, 2 * h + 1
                        po = psum_o.tile([128, QB], f32)
                        pse = psum_se.tile([128, QB], f32)
                        for kb in range(kb_hi + 1):
                            j = kb - qb * (QB // 128)  # >=0 in diagonal zone
                            # j==3 pads the active range to N=256 (fp32r is
                            # 4x slower below 256); the extra below-diagonal
                            # strip is zeroed by the widened [zeros|tri] mask
                            lo = 256 if j == 3 else max(0, j) * 128
                            ps = psum_s.tile([128, QB], f32)
                            nc.tensor.matmul(
                                ps[:, lo:],
                                qkT[:, k_slot, kb * 128:(kb + 1) * 128],
                                qkT[:, q_slot, qb * QB + lo:(qb + 1) * QB],
                                start=True, stop=True)
                            es = exp_pool.tile([128, QB], f32r)
                            nc.scalar.activation(
                                es[:, lo:], ps[:, lo:],
                                mybir.ActivationFunctionType.Exp,
                                scale=sTd[:, kb:kb + 1])
                            if j == 3:
                                nc.vector.tensor_tensor(
                                    es[:, 256:512],
                                    es[:, 256:512].bitcast(f32),
                                    zt[:], mybir.AluOpType.mult)
                            elif j >= 0:
                                nc.vector.tensor_tensor(
                                    es[:, j * 128:(j + 1) * 128],
                                    es[:, j * 128:(j + 1) * 128].bitcast(f32),
                                    tri[:], mybir.AluOpType.mult)
                            nc.tensor.matmul(
                                po[:, lo:], v_sb[:, kb, h * D:(h + 1) * D],
                                es[:, lo:], start=(kb == 0), stop=(kb == kb_hi))
                            nc.tensor.matmul(
                                pse[:, lo:], ones_r[:], es[:, lo:],
                                start=(kb == 0), stop=(kb == kb_hi))
                        rse = rse_pool.tile([128, QB], f32)
                        nc.vector.reciprocal_approx_fast(rse[:], pse[:])
                        nc.vector.tensor_tensor(
                            attnT[:, h, qb * QB:(qb + 1) * QB], po[:], rse[:],
                            mybir.AluOpType.mult)

                def outproj(sb, gs=0, ge=8, borrow=False, act_evac=False):
                    # evacs land in a 2-block staging tile; one DMA per group
                    for g in range(gs, ge):
                        st = out_pool.tile([128, 2, 512], f32, tag="ost")
                        for oi in range(2):
                            ob = g * 2 + oi
                            # the score pool is idle during the final block;
                            # borrow its banks to deepen the psum rotation
                            if borrow and ob % 2 == 0:
                                ps = psum_s.tile([128, QB], f32)
                            else:
                                ps = psum_out.tile([128, 512], f32)
                            for ch in range(HPC):
                                nc.tensor.matmul(
                                    ps[:], woT[:, ch, ob * 128:(ob + 1) * 128],
                                    attnT[:, ch, sb * 512:(sb + 1) * 512],
                                    start=(ch == 0), stop=(ch == HPC - 1))
                            if act_evac or ob % 2 == 0:
                                nc.scalar.copy(st[:, oi], ps[:])
                            else:
                                nc.vector.tensor_copy(st[:, oi], ps[:])
                        nc.sync.dma_start(
                            outT_d[g * 256:(g + 1) * 256,
                                   sb * 512:(sb + 1) * 512]
                            .rearrange("(ob p) t -> p ob t", p=128), st[:])

                # interleave: outproj(qb) emitted after attn(qb+1) h=0 so the
                # pse->recip->attnT chain never sits on PE's critical path
                attn_head(0, 0)
                attn_head(0, 1)
                attn_head(1, 0)
                outproj(0)
                attn_head(1, 1)
                attn_head(2, 0)
                outproj(1)
                attn_head(2, 1)
                attn_head(3, 0)
                outproj(2, 0, 6)
                attn_head(3, 1)
                outproj(2, 6, 8, borrow=True, act_evac=True)
                outproj(3, borrow=True, act_evac=True)
    nc.compile()
    return nc


def get_nc():
    global _CACHED_NC
    if _CACHED_NC is None:
        _CACHED_NC = _build()
    return _CACHED_NC


def make_in_maps(x, wqkv, wo):
    x = np.asarray(x, dtype=np.float32)
    wqkv = np.asarray(wqkv, dtype=np.float32)
    wo = np.asarray(wo, dtype=np.float32)
    xT = np.ascontiguousarray(x.T)
    cst = np.concatenate(
        [np.ones((128, 128), np.float32),
         np.zeros((128, 128), np.float32),
         np.triu(np.ones((128, 128), np.float32)),
         np.eye(128, dtype=np.float32)], axis=1)
    in_maps = []
    for c in range(N_CORES):
        wT = np.ascontiguousarray(wqkv[c * FPC:(c + 1) * FPC].T)
        woT = np.ascontiguousarray(wo[:, c * CPC:(c + 1) * CPC].T)
        in_maps.append({"xT": xT, "wT": wT, "woT": woT, "cst": cst})
    return in_maps


def kernel(x, wqkv, wo):
    nc = get_nc()
    in_maps = make_in_maps(x, wqkv, wo)
    res = None
    for attempt in range(4):
        try:
            res = bass_utils.run_bass_kernel_spmd(
                nc, in_maps, core_ids=list(range(N_CORES)))
            break
        except Exception:
            # transient NRT device wedges have been observed; they recover
            # after a short quiescent period, so back off before retrying
            if attempt == 3:
                raise
            import time
            time.sleep(20 * (attempt + 1))
    outT = np.zeros((H, S), dtype=np.float32)
    for c in range(N_CORES):
        outT += res.results[c]["outT"]
    return np.ascontiguousarray(outT.T)



# revision 28
# speedup vs baseline: 1.1368x; 1.1368x over previous
"""Trainium2 Bass kernel for nn_Attention_30305289240928.

Single-layer causal attention with RMSNorm prologue:
    xn = x * rsqrt(mean(x^2) + eps)           (RMSNorm, no weight)
    qkv = xn @ wqkv.T  -> per-head q, k, v    (16 heads, head_dim 128)
    out = softmax(causal(q k^T / sqrt(128))) v, concat heads, @ wo.T

Sharding: head-parallel tensor parallel over 8 NeuronCores.
Core c owns heads 2c, 2c+1 (wqkv rows c*768:(c+1)*768) and the matching
wo input-columns c*256:(c+1)*256. Each core computes a full-shape partial
of the output projection; the host sums the 8 partials.

Device-side design (v2, fp8/fp16 mixed precision):
  - QKV projection runs on fp8e4m3 DoubleRow matmuls (2 k-tiles per
    instruction at 0.5 cycles/col = 4x fp32r element throughput). Inputs
    are host-quantized into scaled hi/lo pairs (x*16, w*64, hi and lo
    stored at the same scale) and the product is reconstructed with the
    3-term correction Wh@Xh + Wl@Xh + Wh@Xl; the dropped Wl@Xl term is
    ~1e-3 relative.
  - RMSNorm: squares of x-hi on ACT+DVE; per-token sums via 1-column
    transposed-stationary matmuls (~1 PE cycle each instead of a full
    ones-matmul). s lands token-on-partition; the free-dim broadcast
    s_bc (= s/32, folding the fp8 descale) is built with a tiny fp16
    transpose plus 1-partition broadcast matmuls. Both Q and K evict
    with a s_bc multiply, so the exp scale is the constant 1024/sqrt(D)
    and exp can batch over multi-block score groups.
  - Attention is fp16 (same 10-bit mantissa as fp32r but full PE rate at
    any N, enabling exact-causal column trimming). Scores are computed
    transposed, S.T[kt, qt], in [128, 4, 512] PSUM groups; exp is one
    ACT instruction per non-diagonal group. Sum-of-exp uses 1-column
    transposed-stationary matmuls accumulated across key blocks. The
    phase is software-pipelined: PV/sum-exp lag the score group by one,
    and output-projection chunks fill the exp-latency gaps in the PE
    stream.
  - The softmax normalizer (times the fp8 scale 16) is broadcast back to
    [*, qt] via fp16 transpose + 1-partition broadcast matmuls; PV
    output is normalized and fp8 hi/lo-quantized at eviction.
  - Output projection runs fp8 DoubleRow over the two head chunks
    (3-term hi/lo), producing natural-orientation [tok, hid] fp16
    output; the 1/1024 descale rides the eviction, which rotates over
    ACT/DVE/Pool.
"""

import numpy as np
import ml_dtypes

import concourse.bacc as bacc
import concourse.mybir as mybir
import concourse.tile as tile
from concourse import bass_utils

# Problem shapes (hardcoded per contract)
S = 2048          # sequence length
H = 2048          # hidden
NH = 16           # heads
D = 128           # head dim
EPS = 1e-5
N_CORES = 8
HPC = NH // N_CORES        # heads per core = 2
FPC = 3 * D * HPC          # wqkv features per core = 768
CPC = D * HPC              # attn dims (wo input cols) per core = 256

TB = 256                   # token block width (phase 1)
NTB = S // TB              # 8
NP = 8                     # DoubleRow k-tile pairs over H (2048/256)
QB = 512                   # query block width (phase 2)
NKB = S // 128             # 16 key 128-blocks

SX = 16.0                  # fp8 scale for x and attn values
SW = 64.0                  # fp8 scale for wqkv and wo
DESCALE = 1.0 / (SX * SW)  # 1/1024
SQB = 32.0                 # Q/K eviction scale denominator: qk carry s/SQB
SQRT_D = float(np.sqrt(D))
# qkT carries (SX*SW/SQB)*s*Q~, so logits = score_psum/((SX*SW/SQB)^2 sqrt(D))
EXP_SCALE = 1.0 / ((SX * SW / SQB) ** 2 * SQRT_D)

f32 = mybir.dt.float32
f32r = mybir.dt.float32r
f16 = mybir.dt.float16
f8 = mybir.dt.float8e4
DR = mybir.MatmulPerfMode.DoubleRow
MULT = mybir.AluOpType.mult
SUB = mybir.AluOpType.subtract
EXP = mybir.ActivationFunctionType.Exp
SQRT = mybir.ActivationFunctionType.Sqrt
SQUARE = mybir.ActivationFunctionType.Square
COPY = mybir.ActivationFunctionType.Copy

E4M3 = ml_dtypes.float8_e4m3

_CACHED_NC = None


def _build():
    nc = bacc.Bacc("TRN2", target_bir_lowering=False, debug=False,
                   num_devices=N_CORES)
    # x8: [tb, p, hilo, pair, two, t_rel] packed fp8 (hi and lo at x*SX scale)
    x8_d = nc.dram_tensor("x8", [NTB, 128, 2 * NP * 2 * TB], f8,
                          kind="ExternalInput").ap()
    # w8: [pair, p, hilo, two, f'] fp8, f' = [q0|k0|q1|k1|v0|v1] each 128
    w8_d = nc.dram_tensor("w8", [NP, 128, 2 * 2 * FPC], f8,
                          kind="ExternalInput").ap()
    # wo8: [p, hilo, two(head), hid] fp8
    wo8_d = nc.dram_tensor("wo8", [128, 2 * 2 * H], f8,
                           kind="ExternalInput").ap()
    # fp16 consts: [tri(128) | eye(128)]
    cst_d = nc.dram_tensor("cst16", [128, 256], f16, kind="ExternalInput").ap()
    # sum-exp slot selector: sel[n, qb*4+c] = 1 iff slot n belongs to
    # query-chunk c under qb's emission order
    sel_d = nc.dram_tensor("sel16", [64, 16], f16, kind="ExternalInput").ap()
    # natural-orientation fp16 output [tok, hid]
    out_d = nc.dram_tensor("out", [S, H], f16, kind="ExternalOutput").ap()

    with tile.TileContext(nc) as tc:
        with tc.tile_pool(name="const", bufs=1) as const_pool, \
             tc.tile_pool(name="qk", bufs=1) as qk_pool, \
             tc.tile_pool(name="vsb", bufs=1) as v_pool, \
             tc.tile_pool(name="attn8", bufs=1) as attn_pool, \
             tc.tile_pool(name="svec", bufs=1) as s_pool:

            tri16 = const_pool.tile([128, 128], f16, tag="tri")
            eye16 = const_pool.tile([128, 128], f16, tag="eye")
            sel16 = const_pool.tile([64, 16], f16, tag="sel")
            slots_sb = const_pool.tile([128, 64], f16, tag="slots")
            t_sb = const_pool.tile([64, 128], f16, tag="tsb")
            nc.gpsimd.memset(slots_sb[:], 0.0)
            ones_c16 = const_pool.tile([128, 1], f16, tag="oc16")
            row1 = const_pool.tile([1, 128], f16, tag="row1")
            rowSA = const_pool.tile([1, 128], f16, tag="rowSA")
            eps_b = const_pool.tile([128, 1], f32, tag="eps")
            nc.gpsimd.memset(ones_c16[:], 1.0)
            nc.gpsimd.memset(row1[:], 1.0)
            nc.gpsimd.memset(rowSA[:], SX)
            # s chain emits SQB*sqrt(mean x^2 + eps): bias = eps*SQB^2
            nc.gpsimd.memset(eps_b[:], EPS * SQB * SQB)

            # phase-1 outputs (live into phases 2/3)
            qkT = qk_pool.tile([128, 4, S], f16)      # [q0,k0,q1,k1] x S
            v_sb = v_pool.tile([128, NKB, CPC], f16)  # V natural, kt-chunked
            attn8h = attn_pool.tile([128, HPC, S], f8, tag="ah")
            attn8l = attn_pool.tile([128, HPC, S], f8, tag="al")
            s_bc = s_pool.tile([128, NTB, TB], f16)   # s/SQB bcast over parts
            sT = s_pool.tile([128, NKB], f32)         # s/SQB, t on parts

            # ---------------- Phase 1: RMSNorm stats + QKV projection ------
            with tc.tile_pool(name="wt", bufs=1) as wt_pool, \
                 tc.tile_pool(name="xt", bufs=2) as xt_pool, \
                 tc.tile_pool(name="sq", bufs=2) as sq_pool, \
                 tc.tile_pool(name="ph1", bufs=2) as ph1_pool, \
                 tc.tile_pool(name="ps_qk", bufs=4, space="PSUM") as psum_qk, \
                 tc.tile_pool(name="ps_v", bufs=2, space="PSUM") as psum_v, \
                 tc.tile_pool(name="ps_ssq", bufs=1, space="PSUM") as psum_ssq, \
                 tc.tile_pool(name="ps_sbc", bufs=1, space="PSUM") as psum_sbc:

                # w8 per-pair tiles (one DMA per pair for a fast ramp)
                w8 = wt_pool.tile([128, NP, 2, 2, FPC], f8, tag="w8")
                xt_cur = xt_pool.tile([128, 2, NP, 2, TB], f8, tag="xt")
                nc.sync.dma_start(
                    xt_cur[:, 0],
                    x8_d[0, :, 0:NP * 2 * TB]
                    .rearrange("p (j two t) -> p j two t", j=NP, two=2))
                for j in range(NP):
                    nc.sync.dma_start(
                        w8[:, j],
                        w8_d[j].rearrange("p (hl two f) -> p hl two f", hl=2,
                                          two=2))
                    if j == 0:
                        nc.sync.dma_start(
                            xt_cur[:, 1],
                            x8_d[0, :, NP * 2 * TB:]
                            .rearrange("p (j two t) -> p j two t", j=NP, two=2))
                    if j == 2:
                        nc.sync.dma_start(tri16[:], cst_d[:, 0:128])
                        nc.sync.dma_start(eye16[:], cst_d[:, 128:256])
                        nc.sync.dma_start(sel16[:], sel_d)

                for tb in range(NTB):
                    xt = xt_cur
                    if tb + 1 < NTB:
                        xt_next = xt_pool.tile([128, 2, NP, 2, TB], f8,
                                               tag="xt")
                        nc.sync.dma_start(
                            xt_next[:],
                            x8_d[tb + 1].rearrange(
                                "p (hl j two t) -> p hl j two t", hl=2, j=NP,
                                two=2))

                    # squares of x-hi (scaled 16x): half ACT, half DVE
                    sq = sq_pool.tile([128, NP, 2, TB], f16, tag="sq")
                    nc.scalar.activation(
                        sq[:, 0:NP // 2].rearrange("p a b c -> p (a b c)"),
                        xt[:, 0, 0:NP // 2].rearrange("p a b c -> p (a b c)"),
                        SQUARE)
                    nc.vector.tensor_tensor(
                        sq[:, NP // 2:].rearrange("p a b c -> p (a b c)"),
                        xt[:, 0, NP // 2:].rearrange("p a b c -> p (a b c)"),
                        xt[:, 0, NP // 2:].rearrange("p a b c -> p (a b c)"),
                        MULT)

                    def qk_dr(fb, ps):
                        # 3-term hi/lo DoubleRow accumulation for one slot
                        first = True
                        for wsel, xsel in ((0, 0), (1, 0), (0, 1)):
                            for j in range(NP):
                                nc.tensor.matmul(
                                    ps[:], w8[:, j, wsel, :,
                                              fb * 128:(fb + 1) * 128],
                                    xt[:, xsel, j], perf_mode=DR,
                                    start=first,
                                    stop=(wsel, xsel, j) == (0, 1, NP - 1))
                                first = False

                    # K blocks first (squares run on ACT/DVE meanwhile)
                    ps_k0 = psum_qk.tile([128, TB], f32, tag="qkps",
                                         name="ps_k0")
                    qk_dr(1, ps_k0)
                    ps_k1 = psum_qk.tile([128, TB], f32, tag="qkps",
                                         name="ps_k1")
                    qk_dr(3, ps_k1)

                    # per-token sum of squares: 1-col transposed-stationary.
                    # m-outer: PSUM start marks the whole 2KB bank pending-
                    # zero, so accumulation groups sharing a bank must be
                    # strictly sequential, never interleaved.
                    ps_ssq = psum_ssq.tile([128, 4], f32)
                    for m in range(2):
                        for j in range(NP):
                            for two in range(2):
                                nc.tensor.matmul(
                                    ps_ssq[:, m:m + 1],
                                    sq[:, j, two, m * 128:(m + 1) * 128],
                                    ones_c16[:],
                                    start=(j == 0 and two == 0),
                                    stop=(j == NP - 1 and two == 1))
                    # s/SQB = 1/(SQB*sqrt(mean x^2 + eps)); psum holds
                    # SX^2*ssq so scale by SQB^2/(SX^2*H)
                    sqrt_t = ph1_pool.tile([128, 4], f32, tag="sqrt")
                    nc.scalar.activation(sqrt_t[:, 0:2], ps_ssq[:, 0:2], SQRT,
                                         bias=eps_b[:],
                                         scale=SQB * SQB / (SX * SX * H))
                    nc.vector.reciprocal_approx_fast(sT[:, 2 * tb:2 * tb + 2],
                                                     sqrt_t[:, 0:2])
                    s16 = ph1_pool.tile([128, 2], f16, tag="s16")
                    nc.vector.tensor_copy(s16[:], sT[:, 2 * tb:2 * tb + 2])
                    # per-column transposes to partition-0 rows, then
                    # 1-partition broadcast matmuls
                    ps_bc = psum_sbc.tile([128, TB], f32)
                    ps_tr = ps_bc.bitcast(f16)
                    for m in range(2):
                        nc.tensor.transpose(
                            ps_tr[0:1, 256 + m * 128:384 + m * 128],
                            s16[:, m:m + 1], eye16[:])
                    srow = ph1_pool.tile([1, 256], f16, tag="srow")
                    nc.vector.tensor_copy(srow[:], ps_tr[0:1, 256:512])
                    for m in range(2):
                        nc.tensor.matmul(ps_bc[:, m * 128:(m + 1) * 128],
                                         row1[:],
                                         srow[0:1, m * 128:(m + 1) * 128],
                                         start=True, stop=True)
                    nc.scalar.copy(s_bc[:, tb], ps_bc[:])

                    # Q blocks, then all four evictions (x s_bc = s/SQB)
                    ps_q0 = psum_qk.tile([128, TB], f32, tag="qkps",
                                         name="ps_q0")
                    qk_dr(0, ps_q0)
                    ps_q1 = psum_qk.tile([128, TB], f32, tag="qkps",
                                         name="ps_q1")
                    qk_dr(2, ps_q1)
                    for slot, ps in ((1, ps_k0), (3, ps_k1), (0, ps_q0),
                                     (2, ps_q1)):
                        nc.vector.tensor_tensor(
                            qkT[:, slot, tb * TB:(tb + 1) * TB], ps[:],
                            s_bc[:, tb], MULT)

                    # V blocks: out (t, dv); lhsT = x pairs, rhs = wv pairs
                    for m in range(2):
                        ps = psum_v.tile([128, CPC], f32)
                        first = True
                        for xsel, wsel in ((0, 0), (0, 1), (1, 0)):
                            for j in range(NP):
                                nc.tensor.matmul(
                                    ps[:],
                                    xt[:, xsel, j, :, m * 128:(m + 1) * 128],
                                    w8[:, j, wsel, :, 4 * 128:6 * 128],
                                    perf_mode=DR, start=first,
                                    stop=(xsel, wsel, j) == (1, 0, NP - 1))
                                first = False
                        chunk = tb * 2 + m
                        # v = ps * (s/SQB) * (SQB/(SX*SW)) = V~ * s
                        nc.vector.tensor_scalar(
                            v_sb[:, chunk], ps[:], sT[:, chunk:chunk + 1],
                            SQB * DESCALE, MULT, MULT)

                    if tb + 1 < NTB:
                        xt_cur = xt_next

            # -------- Phase 2+3: attention (qb-outer) + output projection ---
            with tc.tile_pool(name="wo", bufs=1) as wo_pool, \
                 tc.tile_pool(name="exps", bufs=2) as exp_pool, \
                 tc.tile_pool(name="rse", bufs=2) as rse_pool, \
                 tc.tile_pool(name="a16", bufs=2) as a16_pool, \
                 tc.tile_pool(name="ostage", bufs=4) as out_pool, \
                 tc.tile_pool(name="ps_s", bufs=1, space="PSUM") as psum_s, \
                 tc.tile_pool(name="ps_po", bufs=1, space="PSUM") as psum_po, \
                 tc.tile_pool(name="ps_op", bufs=2, space="PSUM") as psum_op, \
                 tc.tile_pool(name="ps_sm", bufs=1, space="PSUM") as psum_sm:
                # wo8 streams in while early attention runs
                wo8 = wo_pool.tile([128, 2, 2, H], f8)
                nc.sync.dma_start(
                    wo8[:], wo8_d.rearrange("p (hl two o) -> p hl two o",
                                            hl=2, two=2))

                # deferred output-projection chunks, emitted into PE gaps
                fills = []
                stages = {}

                def outproj_chunk(qc, hb):
                    if qc not in stages:
                        stages[qc] = out_pool.tile([128, 4, QB], f16,
                                                   tag="st", name=f"st{qc}")
                    st = stages[qc]
                    ps = psum_op.tile([128, QB], f32, tag="op")
                    first = True
                    for asel, wsel in ((0, 0), (1, 0), (0, 1)):
                        a8 = attn8h if asel == 0 else attn8l
                        nc.tensor.matmul(
                            ps[:], a8[:, :, qc * 128:(qc + 1) * 128],
                            wo8[:, wsel, :, hb * QB:(hb + 1) * QB],
                            perf_mode=DR, start=first,
                            stop=(asel, wsel) == (0, 1))
                        first = False
                    # Pool cannot read PSUM; rotate ACT(1):DVE(4) for balance
                    if (qc * 4 + hb) % 5 == 0:
                        nc.scalar.activation(st[:, hb], ps[:], COPY,
                                             scale=DESCALE)
                    else:
                        nc.vector.tensor_scalar_mul(st[:, hb], ps[:], DESCALE)
                    if hb == 3:
                        nc.sync.dma_start(
                            out_d[qc * 128:(qc + 1) * 128, :], st[:])
                        del stages[qc]

                def emit_fills(n):
                    for _ in range(min(n, len(fills))):
                        qc, hb = fills.pop(0)
                        outproj_chunk(qc, hb)

                def attn_head(qb, h):
                    q_slot, k_slot = 2 * h, 2 * h + 1
                    kb_hi = qb * 4 + 3
                    po = psum_po.tile([128, QB], f32, tag="po")
                    sm = psum_sm.tile([128, QB], f32, tag="sm")
                    # sum-exp lands as single-shot 1-col matmuls into "slots"
                    # (PSUM bank-granular pending-zero forbids interleaved
                    # accumulation groups in one bank); slots are reduced
                    # per query-chunk later via transpose + selector matmul.
                    slot_n = [0]

                    def pv_group(g, es):
                        diag = g == qb
                        for kr in range(4):
                            kb = 4 * g + kr
                            lo = kr * 128 if diag else 0
                            nc.tensor.matmul(
                                po[:, lo:],
                                v_sb[:, kb, h * D:(h + 1) * D],
                                es[:, kr, lo:],
                                start=(kb == 0), stop=(kb == kb_hi))
                            for c in range(lo // 128, 4):
                                n = slot_n[0]
                                slot_n[0] += 1
                                nc.tensor.matmul(
                                    sm[:, n:n + 1],
                                    es[:, kr, c * 128:(c + 1) * 128],
                                    ones_c16[:], start=True, stop=True)

                    es_prev = None
                    for g in range(qb + 1):
                        ps = psum_s.tile([128, 4, QB], f32, tag="ps")
                        es = exp_pool.tile([128, 4, QB], f16, tag="es")
                        diag = g == qb
                        for kr in range(4):
                            kb = 4 * g + kr
                            lo = kr * 128 if diag else 0
                            nc.tensor.matmul(
                                ps[:, kr, lo:],
                                qkT[:, k_slot, kb * 128:(kb + 1) * 128],
                                qkT[:, q_slot, qb * QB + lo:(qb + 1) * QB],
                                start=True, stop=True)
                        if diag:
                            for kr in range(4):
                                lo = kr * 128
                                nc.scalar.activation(es[:, kr, lo:],
                                                     ps[:, kr, lo:], EXP,
                                                     scale=EXP_SCALE)
                                nc.vector.tensor_tensor(
                                    es[:, kr, lo:lo + 128],
                                    es[:, kr, lo:lo + 128], tri16[:], MULT)
                        else:
                            nc.scalar.activation(es[:], ps[:], EXP,
                                                 scale=EXP_SCALE)
                        if es_prev is not None:
                            pv_group(g - 1, es_prev)
                            emit_fills(3)
                        es_prev = es
                    pv_group(qb, es_prev)
                    emit_fills(2)

                    # reduce slots per query-chunk: evict -> transpose ->
                    # selector matmul -> sum-exp [qt, 4]
                    n = slot_n[0]
                    nc.vector.tensor_copy(slots_sb[:, 0:n], sm[:, 0:n])
                    sm16 = sm.bitcast(f16)
                    nc.tensor.transpose(sm16[0:64, 768:896], slots_sb[:],
                                        eye16[:])
                    nc.vector.tensor_copy(t_sb[:], sm16[0:64, 768:896])
                    nc.tensor.matmul(sm[:, 64:68], t_sb[:],
                                     sel16[:, qb * 4:qb * 4 + 4],
                                     start=True, stop=True)
                    # normalizer: recip -> fp16 -> transpose -> broadcast
                    rse32 = rse_pool.tile([128, 4], f32, tag="r32")
                    nc.vector.reciprocal_approx_fast(rse32[:], sm[:, 64:68])
                    rse16 = rse_pool.tile([128, 4], f16, tag="r16")
                    nc.vector.tensor_copy(rse16[:], rse32[:])
                    for c in range(4):
                        nc.tensor.transpose(
                            sm16[0:1, 256 + c * 128:384 + c * 128],
                            rse16[:, c:c + 1], eye16[:])
                    rrow = rse_pool.tile([1, 512], f16, tag="rrow")
                    nc.vector.tensor_copy(rrow[:], sm16[0:1, 256:768])
                    for c in range(4):
                        nc.tensor.matmul(sm[:, c * 128:(c + 1) * 128],
                                         rowSA[:],
                                         rrow[0:1, c * 128:(c + 1) * 128],
                                         start=True, stop=True)
                    # A = 16 * po / sumexp (fp16), then fp8 hi/lo quantize
                    # (a tensor_tensor may read at most one PSUM input, so
                    # the broadcast normalizer is staged through SBUF)
                    rbc = rse_pool.tile([128, QB], f16, tag="rbc")
                    nc.scalar.copy(rbc[:], sm[:])
                    a16 = a16_pool.tile([128, QB], f16, tag="a16")
                    nc.vector.tensor_tensor(a16[:], po[:], rbc[:], MULT)
                    nc.gpsimd.tensor_copy(
                        attn8h[:, h, qb * QB:(qb + 1) * QB], a16[:])
                    nc.gpsimd.tensor_tensor(
                        attn8l[:, h, qb * QB:(qb + 1) * QB], a16[:],
                        attn8h[:, h, qb * QB:(qb + 1) * QB], SUB)

                for qb in range(4):
                    attn_head(qb, 0)
                    attn_head(qb, 1)
                    fills.extend((qb * 4 + qq, hb)
                                 for qq in range(4) for hb in range(4))
                emit_fills(len(fills))
    nc.compile()
    return nc


def get_nc():
    global _CACHED_NC
    if _CACHED_NC is None:
        _CACHED_NC = _build()
    return _CACHED_NC


def _hilo(a, scale):
    hi = (a * scale).astype(E4M3)
    lo = (a * scale - hi.astype(np.float32)).astype(E4M3)
    return hi, lo


def make_in_maps(x, wqkv, wo):
    x = np.asarray(x, dtype=np.float32)
    wqkv = np.asarray(wqkv, dtype=np.float32)
    wo = np.asarray(wo, dtype=np.float32)

    # x8: [tb, p, hilo, pair, two, t] from xT[h = pair*256 + two*128 + p, t]
    xh, xl = _hilo(np.ascontiguousarray(x.T), SX)
    x8 = np.stack([a.reshape(NP, 2, 128, NTB, TB).transpose(3, 2, 0, 1, 4)
                   for a in (xh, xl)], axis=2)
    x8 = np.ascontiguousarray(x8.reshape(NTB, 128, 2 * NP * 2 * TB))

    cst = np.concatenate(
        [np.triu(np.ones((128, 128), np.float16)),
         np.eye(128, dtype=np.float16)], axis=1)

    # selector: emission order of sum-exp slots per qb (must match attn_head)
    sel = np.zeros((64, 16), dtype=np.float16)
    for qb in range(4):
        n = 0
        for g in range(qb + 1):
            for kr in range(4):
                lo_c = kr if g == qb else 0
                for c in range(lo_c, 4):
                    sel[n, qb * 4 + c] = 1.0
                    n += 1

    in_maps = []
    for c in range(N_CORES):
        wT = wqkv[c * FPC:(c + 1) * FPC].T            # [2048h, 768f]
        # feature permute to [q0|k0|q1|k1|v0|v1]
        perm = np.r_[0:128, 128:256, 384:512, 512:640, 256:384, 640:768]
        wT = np.ascontiguousarray(wT[:, perm])
        wh, wl = _hilo(wT, SW)
        w8 = np.stack([a.reshape(NP, 2, 128, FPC).transpose(0, 2, 1, 3)
                       for a in (wh, wl)], axis=2)    # [j, p, hl, two, f]
        w8 = np.ascontiguousarray(w8.reshape(NP, 128, 2 * 2 * FPC))

        woT = np.ascontiguousarray(wo[:, c * CPC:(c + 1) * CPC].T)  # [256,2048]
        oh, ol = _hilo(woT, SW)
        wo8 = np.stack([a.reshape(2, 128, H).transpose(1, 0, 2)
                        for a in (oh, ol)], axis=1)   # [p, hl, two, o]
        wo8 = np.ascontiguousarray(wo8.reshape(128, 2 * 2 * H))

        in_maps.append({"x8": x8, "w8": w8, "wo8": wo8, "cst16": cst,
                        "sel16": sel})
    return in_maps


def kernel(x, wqkv, wo):
    nc = get_nc()
    in_maps = make_in_maps(x, wqkv, wo)
    res = None
    for attempt in range(4):
        try:
            res = bass_utils.run_bass_kernel_spmd(
                nc, in_maps, core_ids=list(range(N_CORES)))
            break
        except Exception:
            # transient NRT device wedges have been observed; they recover
            # after a short quiescent period, so back off before retrying
            if attempt == 3:
                raise
            import time
            time.sleep(20 * (attempt + 1))
    out = np.zeros((S, H), dtype=np.float32)
    for c in range(N_CORES):
        out += res.results[c]["out"].astype(np.float32)
    return out
